# revision 9
# baseline (speedup 1.0000x reference)
"""Greedy attention-LAP kernel for TRN2 (8 NeuronCores, data-parallel over batch).

Algorithm per batch b (n1=n2=512):
  mask = cols < ncols[b]
  for r in 0..511:
    logits = where(mask, s[b,r,:], -1e30); p = softmax(logits)*mask
    out[b,r,:] = p if r < nrows[b] else 0
    if r < nrows[b]: mask[argmax(logits)] = False

Kernel structure per core (16 batches):
  Phase 1 (sequential over 64 blocks of 8 rows):
    - mask kept as removed-step code q_enc[c] = 2048 - q (0 = never removed),
      PSUM-resident [128,512] f32, replicated over 8 row-groups, updated by
      PE matmul accumulation of per-block scatter deltas.
    - extraction: x = s_block - 2^101*relu(q_enc + (8K-2048)) ; top-8 values
      (max8) + indices (max_index) per row.
    - PE selector matmuls shuffle indices [128,8] -> [16,64] batch-partition.
    - 8 sequential substeps pick first-alive candidate per row; alive mask
      over all 64 candidate slots updated per pick.
    - picks scattered (gpsimd local_scatter) into f16 delta, PE-accumulated
      into q_enc.
  Phase 2 (pipelined, per block): reconstruct per-row mask from q_enc via
    relu(q_enc + (r-2048)), e = exp(s - masked - 12) with ACT accumulated row
    sum, out = e * (1/sum * active).
"""

import os
import sys

import numpy as np

sys.path.insert(0, "/opt/trn_rl_repo")
sys.path.insert(0, "/opt/trn_rl_repo/concourse")

B, N1, N2 = 128, 512, 512
NCORES = 8
BL = 16  # batches per core
NBLK = 64  # blocks of 8 rows
RPB = 8  # rows per block

QNEVER = 2048.0  # q_enc never-removed offset:  q_enc = 2048 - r
BIGP = float(2.0**101)  # mask scale; relu(a*x) = a*relu(x)
EXPB = -12.0  # fixed softmax shift (values are N(0,1); max<7)

_nc_cache = {}


def build_nc():
    import concourse.bass as bass
    import concourse.bacc as bacc
    import concourse.tile as tile
    from concourse import mybir

    f32 = mybir.dt.float32
    f16 = mybir.dt.float16
    i16 = mybir.dt.int16
    u32 = mybir.dt.uint32
    Alu = mybir.AluOpType
    Act = mybir.ActivationFunctionType

    nc = bacc.Bacc(None, target_bir_lowering=False)

    s_in = nc.dram_tensor("s", [BL, N1, N2], f32, kind="ExternalInput")
    act_all = nc.dram_tensor("act_all", [BL, N1], f32, kind="ExternalInput")
    inact_neg = nc.dram_tensor("inact_neg", [BL, N1], f32, kind="ExternalInput")
    rstep_in = nc.dram_tensor("rstep", [BL, N1], f32, kind="ExternalInput")
    iota_in = nc.dram_tensor("iota512", [BL, N2], f16, kind="ExternalInput")
    qinit_in = nc.dram_tensor("qinit", [BL, N2], f16, kind="ExternalInput")
    w8_in = nc.dram_tensor("w8", [BL, 8], f32, kind="ExternalInput")
    rep16_in = nc.dram_tensor("rep16", [BL, 128], f16, kind="ExternalInput")
    selpack_in = nc.dram_tensor("selpack", [128, RPB, BL], f32, kind="ExternalInput")
    biasP1_in = nc.dram_tensor("biasP1", [128, NBLK], f32, kind="ExternalInput")
    biasR2_in = nc.dram_tensor("biasR2", [128, NBLK], f32, kind="ExternalInput")
    actflag_in = nc.dram_tensor("actflag", [128, NBLK], f32, kind="ExternalInput")
    out_dram = nc.dram_tensor("out", [BL, N1, N2], f32, kind="ExternalOutput")

    # phase-1/2 layout: partition p = j*16 + b  (j = row in block, b = batch)
    # manual APs: for block K, partition (j,b) maps to dram row s[b, 8K+j, :]
    def blk_ap(dram_t, K):
        a = dram_t[:]
        return bass.AP(
            tensor=a.tensor,
            offset=a.offset + K * RPB * N2,
            ap=[[N2, RPB], [N1 * N2, BL], [1, N2]],
        )

    s_r = [blk_ap(s_in, K) for K in range(NBLK)]
    out_r = [blk_ap(out_dram, K) for K in range(NBLK)]

    with tile.TileContext(nc) as tc:
        import contextlib

        ctx = contextlib.ExitStack()
        with ctx:
            consts = ctx.enter_context(tc.tile_pool(name="consts", bufs=1))
            s_pool = ctx.enter_context(tc.tile_pool(name="s_pool", bufs=1))
            big = ctx.enter_context(tc.tile_pool(name="big", bufs=3))
            big2 = ctx.enter_context(tc.tile_pool(name="big2", bufs=3))
            outp_pool = ctx.enter_context(tc.tile_pool(name="outp", bufs=3))
            small = ctx.enter_context(tc.tile_pool(name="small", bufs=3))
            psum_q = ctx.enter_context(tc.tile_pool(name="psq", bufs=1, space="PSUM"))
            psum_c = ctx.enter_context(tc.tile_pool(name="psc", bufs=2, space="PSUM"))

            # ---- load constants ----
            def load_const(dram, shape, dtype, tag):
                t = consts.tile(shape, dtype, tag=tag)
                nc.sync.dma_start(out=t, in_=dram[:])
                return t

            c_act = load_const(act_all, [BL, N1], f32, "c_act")
            c_inact = load_const(inact_neg, [BL, N1], f32, "c_inact")
            c_rstep = load_const(rstep_in, [BL, N1], f32, "c_rstep")
            c_iota = load_const(iota_in, [BL, N2], f16, "c_iota")
            c_qinit = load_const(qinit_in, [BL, N2], f16, "c_qinit")
            c_w8 = load_const(w8_in, [BL, 8], f32, "c_w8")
            c_rep16 = load_const(rep16_in, [BL, 128], f16, "c_rep16")
            c_sel = load_const(selpack_in, [128, RPB, BL], f32, "c_sel")
            c_biasP1 = load_const(biasP1_in, [128, NBLK], f32, "c_biasP1")
            c_biasR2 = load_const(biasR2_in, [128, NBLK], f32, "c_biasR2")
            c_actflag = load_const(actflag_in, [128, NBLK], f32, "c_actflag")

            # ---- load s fully resident ----
            s_tiles = []
            for K in range(NBLK):
                st = s_pool.tile([128, N2], f32, tag=f"s{K}")
                nc.sync.dma_start(out=st, in_=s_r[K])
                s_tiles.append(st)

            bias_exp = consts.tile([128, 1], f32)
            nc.vector.memset(bias_exp, EXPB)

            # ---- q_enc PSUM accumulator init ----
            qenc = psum_q.tile([128, N2], f32)
            nc.tensor.matmul(
                qenc[:], c_rep16[:], c_qinit[:], start=True, stop=True,
                skip_group_check=True,
            )

            for K in range(NBLK):
                # ---------- phase 1: extraction ----------
                tpos = big.tile([128, N2], f32, tag="tpos")
                nc.scalar.activation(
                    tpos, qenc[:], Act.Relu,
                    bias=c_biasP1[:, K : K + 1], scale=BIGP,
                )
                x = big.tile([128, N2], f32, tag="x")
                nc.gpsimd.tensor_tensor(
                    out=x, in0=s_tiles[K][:], in1=tpos[:], op=Alu.subtract
                )
                val8 = small.tile([128, 8], f32, tag="val8")
                nc.vector.max(val8, x[:])
                idx8u = small.tile([128, 8], u32, tag="idx8u")
                nc.vector.max_index(idx8u, val8[:], x[:])
                idx8f = small.tile([128, 8], f32, tag="idx8f")
                nc.vector.tensor_copy(idx8f, idx8u[:])

                # ---------- shuffle indices to batch-partition layout ----------
                cand_ps = psum_c.tile([BL, 64], f32, tag="cand")
                for j in range(RPB):
                    nc.tensor.matmul(
                        cand_ps[:, 8 * j : 8 * j + 8],
                        c_sel[:, j, :], idx8f[:],
                        start=True, stop=True, skip_group_check=True,
                    )
                cidx = small.tile([BL, 64], f32, tag="cidx")
                nc.vector.tensor_copy(cidx, cand_ps[:])

                # ---------- resolve 8 rows sequentially ----------
                alive = small.tile([BL, 64], f32, tag="alive")
                nc.vector.memset(alive, 1.0)
                picksF = small.tile([BL, RPB], f32, tag="picksF")
                t2 = small.tile([BL, 8], f32, tag="t2")
                m2 = small.tile([BL, 1], f32, tag="m2")
                oh8 = small.tile([BL, 8], f32, tag="oh8")
                scr = small.tile([BL, 8], f32, tag="scr")
                for j in range(RPB):
                    r = RPB * K + j
                    nc.vector.tensor_tensor(
                        out=t2, in0=alive[:, 8 * j : 8 * j + 8], in1=c_w8[:],
                        op=Alu.mult,
                    )
                    nc.vector.reduce_max(
                        m2, t2[:], axis=mybir.AxisListType.X
                    )
                    nc.vector.tensor_scalar(
                        out=oh8, in0=t2[:], scalar1=m2[:], scalar2=None,
                        op0=Alu.is_equal,
                    )
                    nc.vector.scalar_tensor_tensor(
                        out=scr, in0=oh8[:], scalar=1.0,
                        in1=cidx[:, 8 * j : 8 * j + 8],
                        op0=Alu.mult, op1=Alu.mult,
                        accum_out=picksF[:, j : j + 1],
                    )
                    nc.vector.scalar_tensor_tensor(
                        out=picksF[:, j : j + 1], in0=picksF[:, j : j + 1],
                        scalar=c_act[:, r : r + 1], in1=c_inact[:, r : r + 1],
                        op0=Alu.mult, op1=Alu.add,
                    )
                    nc.vector.scalar_tensor_tensor(
                        out=alive, in0=cidx[:], scalar=picksF[:, j : j + 1],
                        in1=alive[:], op0=Alu.not_equal, op1=Alu.mult,
                    )
                    oh512 = small.tile([BL, N2], f16, tag="oh512")
                    nc.vector.tensor_scalar(
                        out=oh512, in0=c_iota[:],
                        scalar1=picksF[:, j : j + 1],
                        scalar2=c_rstep[:, r : r + 1],
                        op0=Alu.is_equal, op1=Alu.mult,
                    )
                    nc.tensor.matmul(
                        qenc[:], c_rep16[:], oh512[:],
                        start=False, stop=True, skip_group_check=True,
                    )

                # ---------- phase 2 for block K ----------
                tp2 = big2.tile([128, N2], f32, tag="tp2")
                nc.scalar.activation(
                    tp2, qenc[:], Act.Relu,
                    bias=c_biasR2[:, K : K + 1], scale=BIGP,
                )
                x2 = big2.tile([128, N2], f32, tag="x2")
                nc.gpsimd.tensor_tensor(
                    out=x2, in0=s_tiles[K][:], in1=tp2[:], op=Alu.subtract
                )
                e = big2.tile([128, N2], f32, tag="e")
                sumexp = small.tile([128, 1], f32, tag="sumexp")
                nc.scalar.activation(
                    e, x2[:], Act.Exp, bias=bias_exp[:], scale=1.0,
                    accum_out=sumexp,
                )
                rs = small.tile([128, 1], f32, tag="rs")
                nc.vector.reciprocal(rs, sumexp[:])
                rs2 = small.tile([128, 1], f32, tag="rs2")
                nc.vector.tensor_scalar(
                    out=rs2, in0=rs[:], scalar1=c_actflag[:, K : K + 1],
                    scalar2=None, op0=Alu.mult,
                )
                outp = outp_pool.tile([128, N2], f32, tag="outp")
                nc.vector.tensor_scalar(
                    out=outp, in0=e[:], scalar1=rs2[:], scalar2=None,
                    op0=Alu.mult,
                )
                nc.sync.dma_start(out=out_r[K], in_=outp[:])

    nc.compile()
    return nc


def make_tables(nrows, ncols):
    """Host-side per-core constant tables. nrows/ncols: [BL] int arrays."""
    r = np.arange(N1)
    c = np.arange(N2)
    j_of_r = r % RPB

    act = (r[None, :] < nrows[:, None]).astype(np.float32)  # [BL, N1]
    inact = np.where(act > 0, 0.0, -(j_of_r[None, :] + 1.0)).astype(np.float32)
    rstep = np.broadcast_to((QNEVER - r)[None, :], (BL, N1)).astype(np.float32)
    iota512 = np.broadcast_to(c[None, :], (BL, N2)).astype(np.float16)
    qinit = np.where(c[None, :] < ncols[:, None], 0.0, QNEVER + 2.0).astype(
        np.float16
    )
    w8 = np.broadcast_to(np.arange(8, 0, -1, dtype=np.float32)[None, :], (BL, 8))
    rep16 = np.zeros((BL, 128), dtype=np.float16)
    for b in range(BL):
        rep16[b, b::BL] = 1.0
    selpack = np.zeros((128, RPB, BL), dtype=np.float32)
    for j in range(RPB):
        for b in range(BL):
            selpack[BL * j + b, j, b] = 1.0
    Ks = np.arange(NBLK)
    p = np.arange(128)
    biasP1 = np.broadcast_to(
        (RPB * Ks - QNEVER)[None, :] * BIGP, (128, NBLK)
    ).astype(np.float32)
    rowp = RPB * Ks[None, :] + (p // BL)[:, None]  # [128, NBLK] row index
    biasR2 = ((rowp - QNEVER) * BIGP).astype(np.float32)
    actflag = (rowp < nrows[(p % BL)][:, None]).astype(np.float32)
    return {
        "act_all": np.ascontiguousarray(act),
        "inact_neg": np.ascontiguousarray(inact),
        "rstep": np.ascontiguousarray(rstep),
        "iota512": np.ascontiguousarray(iota512),
        "qinit": np.ascontiguousarray(qinit),
        "w8": np.ascontiguousarray(w8.astype(np.float32)),
        "rep16": np.ascontiguousarray(rep16),
        "selpack": np.ascontiguousarray(selpack),
        "biasP1": np.ascontiguousarray(biasP1),
        "biasR2": np.ascontiguousarray(biasR2),
        "actflag": np.ascontiguousarray(actflag),
    }


def kernel(s, nrows, ncols):
    s = np.asarray(s, dtype=np.float32)
    nrows = np.asarray(nrows, dtype=np.int32)
    ncols = np.asarray(ncols, dtype=np.int32)

    if "nc" not in _nc_cache:
        _nc_cache["nc"] = build_nc()
    nc = _nc_cache["nc"]

    from concourse.bass_utils import run_bass_kernel_spmd

    in_maps = []
    for core in range(NCORES):
        lo, hi = core * BL, (core + 1) * BL
        m = {"s": np.ascontiguousarray(s[lo:hi])}
        m.update(make_tables(nrows[lo:hi], ncols[lo:hi]))
        in_maps.append(m)

    res = run_bass_kernel_spmd(
        nc, in_maps, core_ids=list(range(NCORES)),
        trace=bool(int(os.environ.get("LAP_TRACE", "0"))),
    )
    _nc_cache["last_result"] = res
    out = np.concatenate([r["out"] for r in res.results], axis=0)
    return out
